# revision 3
# baseline (speedup 1.0000x reference)
"""GAT kernel for trn2, 8-core SPMD.

Math: nodes = x.transpose(2,0,1,3).reshape(63, 256000); h = nodes @ W;
a_src = h@att_src; a_dst = h@att_dst; e = leaky(a_dst[:,None]+a_src[None,:]);
out = softmax(e,1) @ h + bias, then mean over channels -> (63,1).

Since out.mean(1) = softmax(e) @ (h.mean(1)) + bias.mean(), the full h is
never needed: only three linear functionals of h (a_src, a_dst, hbar).
Each core contracts its shard of (x, W) into a partial (63,3) = h @ P
with P = [att_src, att_dst, ones/256]; one 756-byte AllReduce combines
them; the 63x63 softmax epilogue is computed redundantly on every core.
"""

import numpy as np

A, B, C, D = 1024, 1, 63, 250
IN_CH = A * B * D
OUT_CH = 256
NEG_SLOPE = 0.2
N_CORES = 8
A_PER_CORE = A // N_CORES          # 128
ROWS_PER_CORE = A_PER_CORE * D     # 32000
AB = 16                            # a-values per DMA block
NBLK = A_PER_CORE // AB            # 8

_CACHE = {}
LAST_RESULT = None


def _build():
    import concourse.mybir as mybir
    import concourse.tile as tile
    from concourse import bacc
    from concourse.masks import make_identity

    f32 = mybir.dt.float32
    X = mybir.AxisListType.X
    add = mybir.AluOpType.add
    mult = mybir.AluOpType.mult
    amax = mybir.AluOpType.max

    nc = bacc.Bacc("TRN2", target_bir_lowering=False, debug=False,
                   num_devices=N_CORES)

    xs_d = nc.dram_tensor("xs", [A_PER_CORE, C, D], f32, kind="ExternalInput")
    Ws_d = nc.dram_tensor("Ws", [ROWS_PER_CORE, OUT_CH], f32, kind="ExternalInput")
    asrc_d = nc.dram_tensor("att_src", [2, 128], f32, kind="ExternalInput")
    adst_d = nc.dram_tensor("att_dst", [2, 128], f32, kind="ExternalInput")
    bias_d = nc.dram_tensor("bias", [1, OUT_CH], f32, kind="ExternalInput")
    out_d = nc.dram_tensor("out", [C, 1], f32, kind="ExternalOutput")
    cc_in = nc.dram_tensor("cc_in", [C, 3], f32)
    cc_out = nc.dram_tensor("cc_out", [C, 3], f32, addr_space="Shared")

    with tile.TileContext(nc) as tc:
        with (
            tc.tile_pool(name="const", bufs=1) as constp,
            tc.tile_pool(name="w", bufs=2) as wp,
            tc.tile_pool(name="x", bufs=2) as xp,
            tc.tile_pool(name="xt", bufs=3) as xtp,
            tc.tile_pool(name="pt", bufs=2, space="PSUM") as ptp,
            tc.tile_pool(name="acc", bufs=1, space="PSUM") as accp,
            tc.tile_pool(name="eps", bufs=1, space="PSUM") as epp,
            tc.tile_pool(name="ep", bufs=1) as ep,
        ):
            ident = constp.tile([126, 126], f32)
            make_identity(nc, ident[:, :])

            # hT accumulators: partition = output channel (two halves of 256)
            hT0 = accp.tile([128, C], f32)
            hT1 = accp.tile([128, C], f32)

            for blk in range(NBLK):
                a0 = blk * AB
                # W rows for a in [a0, a0+AB), pair-of-rows per partition:
                # wt[d2, aa, r, o] = W[(a0+aa)*250 + 2*d2 + r, o]
                wt = wp.tile([125, AB, 2, OUT_CH], f32, tag="wt")
                nc.sync.dma_start(
                    out=wt[:, :, :, :],
                    in_=Ws_d[a0 * D:(a0 + AB) * D, :].rearrange(
                        "(aa d two) o -> d aa two o", aa=AB, d=125, two=2),
                )
                # x for the same a's, two a's interleaved on partitions:
                # xt_in[g*63+c, j, d] = x[a0 + 2j + g, c, d]
                xt_in = xp.tile([126, AB // 2, D], f32, tag="xs")
                nc.sync.dma_start(
                    out=xt_in[:, :, :],
                    in_=xs_d[a0:a0 + AB, :, :].rearrange(
                        "(j g) c d -> (g c) j d", j=AB // 2, g=2),
                )
                for grp in range(AB // 4):     # 2 pairs (4 a's) per group
                    pt = ptp.tile([125, 504], f32, tag="pt")
                    for jj in range(2):
                        j = grp * 2 + jj
                        for r in range(2):
                            # (126,125) -> (125,126) transpose via PE
                            nc.tensor.transpose(
                                pt[:, (jj * 2 + r) * 126:(jj * 2 + r + 1) * 126],
                                xt_in[:, j, r:D:2],
                                ident[:, :],
                            )
                    xts = xtp.tile([125, 504], f32, tag="xts")
                    nc.vector.tensor_copy(xts[:, :], pt[:, :])
                    for jj in range(2):
                        j = grp * 2 + jj
                        for g in range(2):
                            aa = j * 2 + g
                            for r in range(2):
                                base = (jj * 2 + r) * 126 + g * 63
                                rhs_x = xts[:, base:base + 63]
                                first = blk == 0 and grp == 0 and jj == 0 and g == 0 and r == 0
                                last = (blk == NBLK - 1 and grp == AB // 4 - 1
                                        and jj == 1 and g == 1 and r == 1)
                                nc.tensor.matmul(hT0[:, :], wt[:, aa, r, 0:128],
                                                 rhs_x, start=first, stop=last)
                                nc.tensor.matmul(hT1[:, :], wt[:, aa, r, 128:256],
                                                 rhs_x, start=first, stop=last)

            # ---- epilogue: project hT -> (63,3) partials ----
            hTs = ep.tile([128, 2, C], f32)
            nc.vector.tensor_copy(hTs[:, 0, :], hT0[:, :])
            nc.vector.tensor_copy(hTs[:, 1, :], hT1[:, :])

            P_sb = ep.tile([128, 2, 3], f32)
            nc.sync.dma_start(out=P_sb[:, :, 0],
                              in_=asrc_d[:, :].rearrange("c p -> p c"))
            nc.sync.dma_start(out=P_sb[:, :, 1],
                              in_=adst_d[:, :].rearrange("c p -> p c"))
            nc.vector.memset(P_sb[:, :, 2], 1.0 / OUT_CH)

            acb_ps = epp.tile([C, 3], f32, tag="acb")
            for c2 in range(2):
                nc.tensor.matmul(acb_ps[:, :], hTs[:, c2, :], P_sb[:, c2, :],
                                 start=c2 == 0, stop=c2 == 1)
            acb_sb = ep.tile([C, 3], f32)
            nc.vector.tensor_copy(acb_sb[:, :], acb_ps[:, :])
            nc.sync.dma_start(out=cc_in[:, :], in_=acb_sb[:, :])

            nc.gpsimd.collective_compute(
                "AllReduce", add,
                replica_groups=[list(range(N_CORES))],
                ins=[cc_in.ap()], outs=[cc_out.ap()],
            )

            acb = ep.tile([C, 3], f32)
            nc.sync.dma_start(out=acb[:, :], in_=cc_out[:, :])

            # rows: a_src and hbar as (1,63) rows via tiny PE transposes
            rows_ps = epp.tile([1, 126], f32, tag="rows")
            nc.tensor.transpose(rows_ps[0:1, 0:63], acb[:, 0:1], ident[0:63, 0:63])
            nc.tensor.transpose(rows_ps[0:1, 63:126], acb[:, 2:3], ident[0:63, 0:63])
            rows = ep.tile([1, 126], f32)
            nc.vector.tensor_copy(rows[0:1, :], rows_ps[0:1, :])

            # hbar' = hbar + mean(bias)
            bt = ep.tile([1, OUT_CH], f32)
            nc.sync.dma_start(out=bt[0:1, :], in_=bias_d[:, :])
            bsum = ep.tile([1, 1], f32)
            nc.vector.reduce_sum(bsum[0:1, :], bt[0:1, :], axis=X)
            nc.vector.tensor_scalar_mul(bsum[0:1, :], bsum[0:1, :], 1.0 / OUT_CH)
            nc.vector.tensor_scalar_add(rows[0:1, 63:126], rows[0:1, 63:126],
                                        bsum[0:1, :])

            # broadcast rows across the 63 node partitions
            asb = ep.tile([C, C], f32)
            nc.gpsimd.partition_broadcast(asb[:, :], rows[0:1, 0:63])
            wbb = ep.tile([C, C], f32)
            nc.gpsimd.partition_broadcast(wbb[:, :], rows[0:1, 63:126])

            # e = leaky_relu(a_dst[i] + a_src[j])
            u = ep.tile([C, C], f32)
            nc.vector.tensor_scalar(u[:, :], asb[:, :], acb[:, 1:2], None, add)
            u2 = ep.tile([C, C], f32)
            nc.vector.tensor_scalar_mul(u2[:, :], u[:, :], NEG_SLOPE)
            e = ep.tile([C, C], f32)
            nc.vector.tensor_tensor(e[:, :], u[:, :], u2[:, :], amax)

            # softmax-weighted sum of hbar'
            nm = ep.tile([C, 1], f32)
            nc.vector.reduce_max(nm[:, :], e[:, :], axis=X, negate=True)
            pexp = ep.tile([C, C], f32)
            s = ep.tile([C, 1], f32)
            nc.scalar.activation(pexp[:, :], e[:, :],
                                 mybir.ActivationFunctionType.Exp,
                                 bias=nm[:, :], scale=1.0, accum_out=s[:, :])
            prod = ep.tile([C, C], f32)
            tsum = ep.tile([C, 1], f32)
            nc.vector.tensor_tensor(prod[:, :], pexp[:, :], wbb[:, :], mult)
            nc.vector.reduce_sum(tsum[:, :], prod[:, :], axis=X)
            rs = ep.tile([C, 1], f32)
            nc.vector.reciprocal(rs[:, :], s[:, :])
            oc = ep.tile([C, 1], f32)
            nc.vector.tensor_tensor(oc[:, :], tsum[:, :], rs[:, :], mult)
            nc.sync.dma_start(out=out_d[:, :], in_=oc[:, :])

    nc.compile()
    return nc


def kernel(x, W, att_src, att_dst, bias, trace=False):
    global LAST_RESULT
    from concourse.bass_utils import run_bass_kernel_spmd

    if "nc" not in _CACHE:
        _CACHE["nc"] = _build()
    nc = _CACHE["nc"]

    x = np.asarray(x, dtype=np.float32)
    W = np.asarray(W, dtype=np.float32)
    att_src = np.asarray(att_src, dtype=np.float32).reshape(2, 128)
    att_dst = np.asarray(att_dst, dtype=np.float32).reshape(2, 128)
    bias = np.asarray(bias, dtype=np.float32).reshape(1, OUT_CH)

    in_maps = []
    for k in range(N_CORES):
        in_maps.append({
            "xs": np.ascontiguousarray(x[k * A_PER_CORE:(k + 1) * A_PER_CORE, 0]),
            "Ws": np.ascontiguousarray(W[k * ROWS_PER_CORE:(k + 1) * ROWS_PER_CORE]),
            "att_src": att_src,
            "att_dst": att_dst,
            "bias": bias,
        })

    res = run_bass_kernel_spmd(nc, in_maps, core_ids=list(range(N_CORES)),
                               trace=trace)
    LAST_RESULT = res
    return res.results[0]["out"]
